# revision 1
# baseline (speedup 1.0000x reference)
"""Trainium2 Bass kernel for DirectConv2D (3x3 VALID, NCHW/OIHW).

Problem: x [32, 256, 56, 56] int32 (values 0..7 after clip),
         weight [256, 256, 3, 3] fp32 (small non-negative ints 0..6)
         -> out [32, 256, 54, 54] fp32.

Strategy:
 - Data-parallel across 8 NeuronCores: 4 images per core, weight replicated.
 - Conv decomposed into 9 shifted matmuls (one per kernel tap) accumulated
   in PSUM; contraction over the 256 input channels.
 - Inputs are tiny non-negative integers, so fp8-e4m3 matmuls are exact
   (products <= 42, fp32 PSUM accumulation). DoubleRow perf mode contracts
   all 256 input channels (2 x 128-partition k-tiles) per matmul.
 - Activations live in SBUF as [128 part, chunk 2, img 4, pix 3140]
   (56*56=3136 pixels + 4 pad so every tile can read a full 504-wide
   window). Output computed in tiles of 9 rows x 56 cols = 504 <= 512
   (one PSUM bank); only the 54 valid cols per row are stored.
"""

import sys

sys.path.insert(0, "/opt/trn_rl_repo")

import ml_dtypes
import numpy as np

N_CORES = 8
IMGS = 4  # images per core
H = W = 56
OH = OW = 54
PIX = H * W  # 3136
PIXP = PIX + 4  # padded so kh=2,kw=2 window of width 504 stays in-bounds
ROWS_PER_TILE = 9
N_TILE = ROWS_PER_TILE * W  # 504 (<= 512 fp32 PSUM bank)
N_ROWTILES = OH // ROWS_PER_TILE  # 6

_PROGRAM_CACHE = {}


def _build_program(mode="fp8dr"):
    import concourse.bacc as bacc
    import concourse.mybir as mybir
    import concourse.tile as tile

    nc = bacc.Bacc(
        "TRN2",
        target_bir_lowering=False,
        debug=False,
        enable_asserts=False,
        num_devices=N_CORES,
    )
    dt8 = mybir.dt.float8e4
    dtb = mybir.dt.bfloat16
    dt_in = dt8 if mode == "fp8dr" else dtb

    x_d = nc.dram_tensor("x_sb", [128, 2, IMGS, PIXP], dt_in, kind="ExternalInput").ap()
    w_d = nc.dram_tensor("w_sb", [128, 2, 9, 2, 128], dt_in, kind="ExternalInput").ap()
    out_d = nc.dram_tensor(
        "out", [IMGS, 256, OH, OW], mybir.dt.float32, kind="ExternalOutput"
    ).ap()

    with tile.TileContext(nc) as tc:
        with (
            tc.tile_pool(name="const", bufs=1) as const_pool,
            tc.tile_pool(name="psum", bufs=6, space="PSUM") as psum_pool,
            tc.tile_pool(name="outs", bufs=4) as out_pool,
        ):
            wt = const_pool.tile([128, 2, 9, 2, 128], dt_in)
            nc.sync.dma_start(out=wt, in_=w_d)
            xt = const_pool.tile([128, 2, IMGS, PIXP], dt_in)
            for c in range(2):
                for n in range(IMGS):
                    nc.sync.dma_start(out=xt[:, c, n], in_=x_d[:, c, n])

            for n in range(IMGS):
                for oc in range(2):
                    for t in range(N_ROWTILES):
                        h0 = t * ROWS_PER_TILE
                        pt = psum_pool.tile([128, N_TILE], mybir.dt.float32)
                        k = 0
                        for kh in range(3):
                            for kw in range(3):
                                off = (h0 + kh) * W + kw
                                if mode == "fp8dr":
                                    nc.tensor.matmul(
                                        pt,
                                        wt[:, oc, k, :, :],
                                        xt[:, :, n, off : off + N_TILE],
                                        start=(k == 0),
                                        stop=(k == 8),
                                        perf_mode=mybir.MatmulPerfMode.DoubleRow,
                                    )
                                else:
                                    for c in range(2):
                                        nc.tensor.matmul(
                                            pt,
                                            wt[:, oc, k, c, :],
                                            xt[:, c, n, off : off + N_TILE],
                                            start=(k == 0 and c == 0),
                                            stop=(k == 8 and c == 1),
                                        )
                                k += 1
                        ot = out_pool.tile([128, N_TILE], mybir.dt.float32)
                        nc.vector.tensor_copy(out=ot, in_=pt)
                        src = ot.rearrange("p (r c) -> p r c", c=W)[:, :, 0:OW]
                        nc.sync.dma_start(
                            out=out_d[n, oc * 128 : (oc + 1) * 128, h0 : h0 + ROWS_PER_TILE, :],
                            in_=src,
                        )
    nc.compile()
    return nc


def get_program(mode="fp8dr"):
    if mode not in _PROGRAM_CACHE:
        _PROGRAM_CACHE[mode] = _build_program(mode)
    return _PROGRAM_CACHE[mode]


def _np_dtype(mode):
    return ml_dtypes.float8_e4m3 if mode == "fp8dr" else ml_dtypes.bfloat16


def prep_weight(weight, mode="fp8dr"):
    """weight [256, 256, 3, 3] OIHW fp32 -> w_sb [128 ki, 2 oc, 9 tap, 2 c, 128 m]."""
    wq = weight.astype(np.int32).astype(np.float32)
    wq = wq.reshape(2, 128, 2, 128, 3, 3)  # [oc, m, c, ki, kh, kw]
    w_sb = np.ascontiguousarray(wq.transpose(3, 0, 4, 5, 2, 1))  # [ki, oc, kh, kw, c, m]
    w_sb = w_sb.reshape(128, 2, 9, 2, 128)
    return w_sb.astype(_np_dtype(mode))


def prep_x_core(x_core, mode="fp8dr"):
    """x_core [IMGS, 256, 56, 56] int32 -> x_sb [128 ki, 2 c, IMGS, PIXP]."""
    xq = np.clip(x_core.astype(np.int32), 0, 7).astype(np.float32)
    xq = xq.reshape(IMGS, 2, 128, PIX)  # [n, c, ki, pix]
    x_sb = np.zeros((128, 2, IMGS, PIXP), np.float32)
    x_sb[:, :, :, :PIX] = xq.transpose(2, 1, 0, 3)
    return x_sb.astype(_np_dtype(mode))


def make_in_maps(x, weight, mode="fp8dr"):
    w_sb = prep_weight(weight, mode)
    return [
        {"x_sb": prep_x_core(x[c * IMGS : (c + 1) * IMGS], mode), "w_sb": w_sb}
        for c in range(N_CORES)
    ]


def kernel(x, weight):
    from concourse.bass_utils import run_bass_kernel_spmd

    mode = "fp8dr"
    nc = get_program(mode)
    in_maps = make_in_maps(np.asarray(x), np.asarray(weight), mode)
    res = run_bass_kernel_spmd(nc, in_maps, list(range(N_CORES)))
    return np.concatenate(
        [res.results[c]["out"] for c in range(N_CORES)], axis=0
    ).astype(np.float32)


# revision 2
# speedup vs baseline: 1.6564x; 1.6564x over previous
"""Trainium2 Bass kernel for DirectConv2D (3x3 VALID, NCHW/OIHW).

Problem: x [32, 256, 56, 56] int32 (values 0..7 after clip),
         weight [256, 256, 3, 3] fp32 (small non-negative ints 0..6)
         -> out [32, 256, 54, 54] fp32.

Strategy:
 - Data-parallel across 8 NeuronCores: 4 images per core, weight replicated.
 - Conv decomposed into 9 shifted matmuls (one per kernel tap) accumulated
   in PSUM; contraction over the 256 input channels.
 - Inputs are tiny non-negative integers, so fp8-e4m3 matmuls are exact
   (products <= 42, fp32 PSUM accumulation). DoubleRow perf mode contracts
   all 256 input channels (2 x 128-partition k-tiles) per matmul.
 - Activations live in SBUF as [128 part, chunk 2, img 4, pix 3140]
   (56*56=3136 pixels + 4 pad so every tile can read a full 504-wide
   window). Output computed in tiles of 9 rows x 56 cols = 504 <= 512
   (one PSUM bank); only the 54 valid cols per row are stored.
"""

import sys

sys.path.insert(0, "/opt/trn_rl_repo")

import ml_dtypes
import numpy as np

N_CORES = 8
IMGS = 4  # images per core
H = W = 56
OH = OW = 54
PIX = H * W  # 3136
PIXP = PIX + 4  # padded so kh=2,kw=2 window of width 504 stays in-bounds
ROWS_PER_TILE = 9
N_TILE = ROWS_PER_TILE * W  # 504 (<= 512 fp32 PSUM bank)
N_ROWTILES = OH // ROWS_PER_TILE  # 6

_PROGRAM_CACHE = {}


def _build_program(mode="fp8dr"):
    import concourse.bacc as bacc
    import concourse.mybir as mybir
    import concourse.tile as tile

    nc = bacc.Bacc(
        "TRN2",
        target_bir_lowering=False,
        debug=False,
        enable_asserts=False,
        num_devices=N_CORES,
    )
    dt8 = mybir.dt.float8e4
    dtb = mybir.dt.bfloat16
    dt_in = dt8 if mode == "fp8dr" else dtb

    x_d = nc.dram_tensor("x_sb", [128, 2, IMGS, PIXP], dt_in, kind="ExternalInput").ap()
    w_d = nc.dram_tensor("w_sb", [128, 2, 9, 2, 128], dt_in, kind="ExternalInput").ap()
    out_d = nc.dram_tensor(
        "out", [IMGS, 256, OH, OW], mybir.dt.float32, kind="ExternalOutput"
    ).ap()

    with tile.TileContext(nc) as tc:
        with (
            tc.tile_pool(name="const", bufs=1) as const_pool,
            tc.tile_pool(name="psum", bufs=8, space="PSUM") as psum_pool,
            tc.tile_pool(name="outs", bufs=3) as out_pool,
        ):
            wt = const_pool.tile([128, 2, 9, 2, 128], dt_in)
            nc.sync.dma_start(out=wt, in_=w_d)
            xt = const_pool.tile([128, 2, IMGS, PIXP], dt_in)
            # n-major order so image 0 (both channel chunks) lands first and
            # the first matmul group can start while later images stream in.
            for n in range(IMGS):
                for c in range(2):
                    nc.sync.dma_start(out=xt[:, c, n], in_=x_d[:, c, n])

            for n in range(IMGS):
                for oc in range(2):
                    # staging for a full (n, oc) output block: 54 rows x 54
                    # cols, dense, so the store is one 1.5MB DMA with
                    # 11.7KB-contiguous lines per partition.
                    ot = out_pool.tile([128, OH * OW], mybir.dt.float32)
                    for t in range(N_ROWTILES):
                        h0 = t * ROWS_PER_TILE
                        pt = psum_pool.tile([128, N_TILE], mybir.dt.float32)
                        k = 0
                        for kh in range(3):
                            for kw in range(3):
                                off = (h0 + kh) * W + kw
                                if mode == "fp8dr":
                                    nc.tensor.matmul(
                                        pt,
                                        wt[:, oc, k, :, :],
                                        xt[:, :, n, off : off + N_TILE],
                                        start=(k == 0),
                                        stop=(k == 8),
                                        perf_mode=mybir.MatmulPerfMode.DoubleRow,
                                    )
                                else:
                                    for c in range(2):
                                        nc.tensor.matmul(
                                            pt,
                                            wt[:, oc, k, c, :],
                                            xt[:, c, n, off : off + N_TILE],
                                            start=(k == 0 and c == 0),
                                            stop=(k == 8 and c == 1),
                                        )
                                k += 1
                        # compact 56-col rows (2 junk cols) to dense 54-col
                        # rows while evacuating PSUM.
                        src = pt.rearrange("p (r c) -> p r c", c=W)[:, :, 0:OW]
                        dst = ot[:, t * ROWS_PER_TILE * OW : (t + 1) * ROWS_PER_TILE * OW]
                        nc.vector.tensor_copy(
                            out=dst.rearrange("p (r c) -> p r c", c=OW), in_=src
                        )
                    nc.sync.dma_start(
                        out=out_d[n, oc * 128 : (oc + 1) * 128, :, :],
                        in_=ot.rearrange("p (h w) -> p h w", w=OW),
                    )
    nc.compile()
    return nc


def get_program(mode="fp8dr"):
    if mode not in _PROGRAM_CACHE:
        _PROGRAM_CACHE[mode] = _build_program(mode)
    return _PROGRAM_CACHE[mode]


def _np_dtype(mode):
    return ml_dtypes.float8_e4m3 if mode == "fp8dr" else ml_dtypes.bfloat16


def prep_weight(weight, mode="fp8dr"):
    """weight [256, 256, 3, 3] OIHW fp32 -> w_sb [128 ki, 2 oc, 9 tap, 2 c, 128 m]."""
    wq = weight.astype(np.int32).astype(np.float32)
    wq = wq.reshape(2, 128, 2, 128, 3, 3)  # [oc, m, c, ki, kh, kw]
    w_sb = np.ascontiguousarray(wq.transpose(3, 0, 4, 5, 2, 1))  # [ki, oc, kh, kw, c, m]
    w_sb = w_sb.reshape(128, 2, 9, 2, 128)
    return w_sb.astype(_np_dtype(mode))


def prep_x_core(x_core, mode="fp8dr"):
    """x_core [IMGS, 256, 56, 56] int32 -> x_sb [128 ki, 2 c, IMGS, PIXP]."""
    xq = np.clip(x_core.astype(np.int32), 0, 7).astype(np.float32)
    xq = xq.reshape(IMGS, 2, 128, PIX)  # [n, c, ki, pix]
    x_sb = np.zeros((128, 2, IMGS, PIXP), np.float32)
    x_sb[:, :, :, :PIX] = xq.transpose(2, 1, 0, 3)
    return x_sb.astype(_np_dtype(mode))


def make_in_maps(x, weight, mode="fp8dr"):
    w_sb = prep_weight(weight, mode)
    return [
        {"x_sb": prep_x_core(x[c * IMGS : (c + 1) * IMGS], mode), "w_sb": w_sb}
        for c in range(N_CORES)
    ]


def kernel(x, weight):
    from concourse.bass_utils import run_bass_kernel_spmd

    mode = "fp8dr"
    nc = get_program(mode)
    in_maps = make_in_maps(np.asarray(x), np.asarray(weight), mode)
    res = run_bass_kernel_spmd(nc, in_maps, list(range(N_CORES)))
    return np.concatenate(
        [res.results[c]["out"] for c in range(N_CORES)], axis=0
    ).astype(np.float32)
